# revision 5
# baseline (speedup 1.0000x reference)
"""AttentionSynapse kernel for Trainium2 (8 NeuronCores, SPMD).

reference math:
    wq_n = wq / ||wq||_e ; wk_n = wk / ||wq||_e          (both normed by wq's norm)
    q = gq @ wq_n ; k = gk @ wk_n                        [b,h,t,z]
    a = (q @ k^T) / sqrt(z), diag -> -inf
    out = logsumexp(a, axis=-1)                          [b,h,t]

kernel math (per core: one batch b, 4 heads):
    s_z   = 1 / (8 * sum_e wq[h,e,z]^2)                  (fold norm^2 and 1/sqrt(64) into k)
    qT    = (gq @ wq)^T          [z,t]  (bf16, 2 heads packed into 128 partitions)
    kTs   = (gk @ wk)^T * s_z    [z,t]
    S     = qT^T @ kTs           [t,s]  per 128-row t-tile, f32 in PSUM
    sums[t]  = sum_s exp(S[t,s])       (ACT exp with fused accum)
    diag[t]  = S[t,t]                  (DVE mul-by-identity + reduce)
    out[t]   = ln(sums[t] - exp(diag[t]))

dataflow per core:
    weights (SWDGE) -> norms on ACT/DVE/PE
    g cast fp32->bf16 (SWDGE, DRAM->DRAM), gk first
    transpose-DMA bf16 (HWDGE ring does ONLY transposes) -> gT [e,t] sbuf
    projections (PE, 2 heads packed in M=128 / row-tiled K=64 scores)
    scores -> ACT exp+accum -> per-head finalize -> PE-transpose -> out DMA
"""

import numpy as np

T = 2048
E = 1024
Z = 64
HLOC = 4  # heads per core
NCORES = 8

_CACHE = {}


def _build():
    """Build + compile the per-core Bass program (same program on all cores)."""
    from contextlib import ExitStack

    import concourse.bass as bass
    import concourse.mybir as mybir
    import concourse.tile as tile
    from concourse import bacc

    fp32 = mybir.dt.float32
    bf16 = mybir.dt.bfloat16
    AF = mybir.ActivationFunctionType
    ALU = mybir.AluOpType

    nc = bacc.Bacc(
        "TRN2",
        target_bir_lowering=False,
        debug=False,
        enable_asserts=False,
        num_devices=NCORES,
    )

    # inputs: g[0]=gk, g[1]=gq (gk first: kTs is on the critical path)
    # w[0]=wq, w[1]=wk, packed [e, 4*64] with col = hl*64 + z (hl = local head)
    g = nc.dram_tensor("g", [2, T, E], fp32, kind="ExternalInput").ap()
    w = nc.dram_tensor("w", [2, E, HLOC * Z], fp32, kind="ExternalInput").ap()
    ident = nc.dram_tensor("ident", [128, 128], fp32, kind="ExternalInput").ap()
    # output stored [hl, p, i] with t = i*128 + p; host transposes on unshard
    o = nc.dram_tensor("o", [HLOC, 128, T // 128], fp32, kind="ExternalOutput").ap()

    with tile.TileContext(nc) as tc, ExitStack() as ctx:
        persist = ctx.enter_context(tc.tile_pool(name="persist", bufs=1))
        dram = ctx.enter_context(tc.tile_pool(name="dram", bufs=1, space="DRAM"))

        ident_sb = persist.tile([128, 128], fp32, tag="ident", name="ident_sb")
        nc.gpsimd.dma_start(ident_sb[:], ident[:])
        eights = persist.tile([128, 1], fp32, tag="eights", name="eights")
        nc.gpsimd.memset(eights[:], 8.0)

        # ---------------- weights: load (SWDGE, before casts), square-sum, cast
        w_bf = {}
        s_col = {}
        with (
            tc.tile_pool(name="wtmp", bufs=1) as wtmp,
            tc.tile_pool(name="psum_a", bufs=2, space="PSUM") as psum_a,
        ):
            ssq = wtmp.tile([128, 256], fp32, tag="ssq", name="ssq")
            for wt in range(2):
                for a in range(8):
                    wf = wtmp.tile(
                        [128, 256], fp32, tag=f"wf{wt}_{a}", name=f"wf{wt}_{a}"
                    )
                    nc.gpsimd.dma_start(wf[:], w[wt, a * 128 : (a + 1) * 128, :])
                    wb = persist.tile(
                        [128, 256], bf16, tag=f"wb{wt}_{a}", name=f"wb{wt}_{a}"
                    )
                    nc.vector.tensor_copy(wb[:], wf[:])
                    w_bf[wt, a] = wb
                    if wt == 0:
                        if a == 0:
                            nc.scalar.activation(ssq[:], wf[:], AF.Square)
                        else:
                            sq = wtmp.tile(
                                [128, 256], fp32, tag=f"sq{a}", name=f"sq{a}"
                            )
                            nc.scalar.activation(sq[:], wf[:], AF.Square)
                            nc.vector.tensor_add(ssq[:], ssq[:], sq[:])
            # n2col[p] = 8 * sum_e wq[e, pg*128+p]^2   (via matmul with 8.0-vector)
            for pg in range(2):
                n2p = psum_a.tile([128, 1], fp32, tag="n2p", name="n2p")
                nc.tensor.matmul(
                    n2p[:],
                    ssq[:, pg * 128 : (pg + 1) * 128],
                    eights[:],
                    start=True,
                    stop=True,
                )
                s_sb = persist.tile([128, 1], fp32, tag=f"scol{pg}", name=f"scol{pg}")
                nc.vector.reciprocal(s_sb[:], n2p[:])
                s_col[pg] = s_sb

        # ---------------- G: cast->bf16 (DRAM), transpose-DMA, project -------
        qT = {}
        kTs = {}
        for pg in range(2):
            qT[pg] = persist.tile([128, T], bf16, tag=f"qT{pg}", name=f"qT{pg}")
            kTs[pg] = persist.tile([128, T], bf16, tag=f"kTs{pg}", name=f"kTs{pg}")
        gT = {}
        for gi in range(2):
            for a in range(8):
                gT[gi, a] = persist.tile(
                    [128, T], bf16, tag=f"gT{gi}_{a}", name=f"gT{gi}_{a}"
                )

        with tc.tile_pool(name="psum_p", bufs=4, space="PSUM") as psum_p:
            for gi in range(2):  # 0 = gk, 1 = gq
                wt = 1 - gi  # gk uses wk=w[1], gq uses wq=w[0]
                for half in range(2):
                    gbf = dram.tile(
                        [T // 2, E],
                        bf16,
                        tag=f"gbf{gi}_{half}",
                        name=f"gbf{gi}_{half}",
                    )
                    for qtr in range(2):
                        r0 = half * 1024 + qtr * 512
                        nc.gpsimd.dma_start(
                            gbf[qtr * 512 : (qtr + 1) * 512, :],
                            g[gi, r0 : r0 + 512, :],
                        )
                    for a in range(8):
                        nc.sync.dma_start(
                            gT[gi, a][:, half * 1024 : (half + 1) * 1024],
                            gbf[:, a * 128 : (a + 1) * 128],
                            transpose=True,
                        )
                    for tq in (2 * half, 2 * half + 1):
                        for pg in range(2):
                            acc = psum_p.tile([128, 512], fp32, tag="acc", name="acc")
                            for a in range(8):
                                nc.tensor.matmul(
                                    acc[:],
                                    w_bf[wt, a][:, pg * 128 : (pg + 1) * 128],
                                    gT[gi, a][:, tq * 512 : (tq + 1) * 512],
                                    start=(a == 0),
                                    stop=(a == 7),
                                )
                            dst = (kTs if gi == 0 else qT)[pg][
                                :, tq * 512 : (tq + 1) * 512
                            ]
                            if gi == 0:
                                nc.vector.tensor_scalar_mul(dst, acc[:], s_col[pg][:])
                            else:
                                nc.vector.tensor_copy(dst, acc[:])

        # ---------------- scores + exp-accum + diag + per-head finalize ------
        with (
            tc.tile_pool(name="psum_s", bufs=2, space="PSUM") as psum_s,
            tc.tile_pool(name="esc", bufs=2) as esc_pool,
            tc.tile_pool(name="prodp", bufs=2) as prod_pool,
            tc.tile_pool(name="headp", bufs=2) as head_pool,
        ):
            for pg in range(2):
                for hh in range(2):
                    hl = pg * 2 + hh
                    sums = head_pool.tile(
                        [128, 16], fp32, tag=f"sums{hl}", name=f"sums{hl}"
                    )
                    diagT = head_pool.tile(
                        [128, 16], fp32, tag=f"diagT{hl}", name=f"diagT{hl}"
                    )
                    for i in range(16):
                        sc = psum_s.tile([128, T], fp32, tag="sc", name="sc")
                        for sq4 in range(4):
                            nc.tensor.matmul(
                                sc[:, sq4 * 512 : (sq4 + 1) * 512],
                                qT[pg][
                                    hh * 64 : (hh + 1) * 64, i * 128 : (i + 1) * 128
                                ],
                                kTs[pg][
                                    hh * 64 : (hh + 1) * 64,
                                    sq4 * 512 : (sq4 + 1) * 512,
                                ],
                                start=True,
                                stop=True,
                                tile_position=(hh * 64, 0),
                            )
                        prod = prod_pool.tile([128, 128], fp32, tag="prod", name="prod")
                        nc.vector.tensor_mul(
                            prod[:], sc[:, i * 128 : (i + 1) * 128], ident_sb[:]
                        )
                        nc.vector.tensor_reduce(
                            diagT[:, i : i + 1],
                            prod[:],
                            axis=mybir.AxisListType.X,
                            op=ALU.add,
                        )
                        esc = esc_pool.tile([128, T], bf16, tag="esc", name="esc")
                        nc.scalar.activation(
                            esc[:], sc[:], AF.Exp, accum_out=sums[:, i : i + 1]
                        )
                    # -------- finalize this head: ln(sums - exp(diag)) -------
                    expd = head_pool.tile(
                        [128, 16], fp32, tag=f"expd{hl}", name=f"expd{hl}"
                    )
                    nc.scalar.activation(expd[:], diagT[:], AF.Exp)
                    corr = head_pool.tile(
                        [128, 16], fp32, tag=f"corr{hl}", name=f"corr{hl}"
                    )
                    nc.vector.tensor_sub(corr[:], sums[:], expd[:])
                    logt = head_pool.tile(
                        [128, 16], fp32, tag=f"logt{hl}", name=f"logt{hl}"
                    )
                    nc.scalar.activation(logt[:], corr[:], AF.Ln)
                    nc.sync.dma_start(o[hl], logt[:])

    nc.compile()
    return nc


def _get_nc():
    if "nc" not in _CACHE:
        _CACHE["nc"] = _build()
    return _CACHE["nc"]


def make_in_maps(gq, gk, wq, wk):
    """Host-side sharding: core i -> batch i//4, heads 4*(i%4) .. +4."""
    ident = np.eye(128, dtype=np.float32)
    in_maps = []
    for core in range(NCORES):
        b = core // 4
        h0 = HLOC * (core % 4)
        # pack heads into columns: [e, hl*64+z]
        wq_c = np.ascontiguousarray(
            wq[h0 : h0 + HLOC].transpose(1, 0, 2).reshape(E, HLOC * Z)
        )
        wk_c = np.ascontiguousarray(
            wk[h0 : h0 + HLOC].transpose(1, 0, 2).reshape(E, HLOC * Z)
        )
        in_maps.append(
            {
                "g": np.ascontiguousarray(np.stack([gk[b], gq[b]])),
                "w": np.ascontiguousarray(np.stack([wq_c, wk_c])),
                "ident": ident,
            }
        )
    return in_maps


def kernel(gq, gk, wq, wk, _trace=False):
    from concourse import bass_utils

    nc = _get_nc()
    in_maps = make_in_maps(gq, gk, wq, wk)
    res = bass_utils.run_bass_kernel_spmd(
        nc, in_maps, core_ids=list(range(NCORES)), trace=_trace
    )
    if _trace:
        _CACHE["last_results"] = res
    b_h = gq.shape[0]
    h = wq.shape[0]
    out = np.empty((b_h, h, T), dtype=np.float32)
    for core in range(NCORES):
        b = core // 4
        h0 = HLOC * (core % 4)
        oc = res.results[core]["o"]  # [HLOC, 128, 16], t = i*128 + p
        for hl in range(HLOC):
            out[b, h0 + hl, :] = oc[hl].T.ravel()
    return out
